# revision 30
# baseline (speedup 1.0000x reference)
"""Trainium2 Bass kernel for nn_AttentionPropagationLayer (GNN message passing).

Strategy (8 NeuronCores, SPMD single program, fp8 message path / bf16 update):
  - Host: build the *directed* edge list (each undirected edge contributes its
    message to both endpoints), bucket directed edges by destination-node
    window (128 nodes), assign the 512 windows to 8 cores x 64 slots
    load-balanced so every core's slot j has the same padded tile count C[j].
    All per-edge data is pre-gathered on the host into ONE dense fp8 stream
    (per 4-tile group: v0-states | v1-states | edge-features | dest one-hots,
    feature-major) - the device does NO gathers, no parity selects, and only
    one big DMA per slot.
  - Device: 3-layer message MLP entirely in fp8 DoubleRow (weights pre-scaled
    by 32 on host, de-scaled for free inside each ReLU's scale factor).
    Layer 2 emits edge-major h2; the per-window segment-sum is the matmul
    G[h2,n] += h2_tile^T @ onehot_tile (DoubleRow over tile pairs), so layer 3
    never materializes per-edge messages: the update-MLP `summed` chunk is
    folded as (W3 @ uW1_mid)^T @ G. Two slots share one G PSUM bank (4
    sub-regions; only the very first matmul uses start=True - later
    sub-regions self-zero via the bank's pending-zero state).
  - PSUM->SBUF ReLU/cast traffic (the binding resource: GPSIMD cannot touch
    PSUM on TRN2, so only ACT and DVE qualify) is split between ACT and DVE
    by a least-finish-time router with size affinity; Pool handles the
    SBUF-only attention subtraction. L1 works on 4-tile groups so its ReLU
    runs as one 1024-col op.
  - Emission is software-pipelined with per-stage iteration offsets
    STAGE_OFFSETS (L1mm | L1relu | L2mm | L2relu | G); the per-PAIR update
    MLP (bf16, on [win_states; G-fold; attention]) is emitted in 3 delayed
    phases so it never starves the message-pipeline engines.

kernel(**inputs) takes the full unsharded inputs (keys as in setup_inputs())
and returns the full [N, D] float32 output.
"""

import sys

for _p in ("/opt/trn_rl_repo", "/root/.axon_site/_ro/trn_rl_repo"):
    if _p not in sys.path:
        sys.path.append(_p)

import numpy as np
import ml_dtypes

import concourse.bass as bass
import concourse.mybir as mybir
import concourse.tile as tile
from concourse import bacc
from concourse.bass_utils import run_bass_kernel_spmd

# ---------------------------------------------------------------- constants
NCORES = 8
P = 128
NUM_NODES_PER_GRAPH = 2048  # reference NUM_NODES (attention pairing)

FT = mybir.dt.float32
BT = mybir.dt.bfloat16
F8 = mybir.dt.float8e4
NP_BT = ml_dtypes.bfloat16
NP_F8 = ml_dtypes.float8_e4m3

D = 128
ED = 64
H = 256
M = 128
U = 256
STAGE_OFFSETS = (0, 2, 6, 8, 10)
SW = 32.0  # power-of-2 pre-scale for fp8 weights (de-scaled in ReLU scale)
XCOLS = 512  # per-tile stream bytes/partition: v0 | v1 | edge(pad) | onehot


def _cdiv(a, b):
    return -(-a // b)


# ---------------------------------------------------------------- host prep
def _preprocess(node_states, edges, vertices):
    """Window binning + the dense pre-gathered per-core streams."""
    node_states = np.asarray(node_states, np.float32)
    edges_np = np.asarray(edges, np.float32)
    N, d = node_states.shape
    E, ed = edges_np.shape
    assert d == D and ed == ED
    NW = N // P
    SLOTS = NW // NCORES
    assert NW % NCORES == 0

    v0 = np.asarray(vertices[:, 0]).astype(np.int64)
    v1 = np.asarray(vertices[:, 1]).astype(np.int64)
    dst = np.concatenate([v0, v1])
    ev0 = np.concatenate([v0, v0])
    ev1 = np.concatenate([v1, v1])
    eid = np.concatenate([np.arange(E), np.arange(E)]).astype(np.int64)

    win = dst // P
    order = np.argsort(win, kind="stable")
    fills = np.bincount(win, minlength=NW).astype(np.int64)
    starts = np.zeros(NW + 1, np.int64)
    starts[1:] = np.cumsum(fills)

    # windows ranked by fill, grouped in NCORES so per-slot padded counts match
    rank = np.argsort(-fills, kind="stable")
    C = np.zeros(SLOTS, np.int64)
    assign = np.zeros((NCORES, SLOTS), np.int64)
    for j in range(SLOTS):
        grp = rank[j * NCORES : (j + 1) * NCORES]
        assign[:, j] = grp
        C[j] = max(1, _cdiv(int(fills[grp].max()), P))
    base = np.zeros(SLOTS + 1, np.int64)
    base[1:] = np.cumsum(C)
    TT = int(C.sum())

    pw = NUM_NODES_PER_GRAPH // P  # partner window = w ^ pw
    ns8 = node_states.astype(NP_F8)
    ed8 = edges_np.astype(NP_F8)
    nsbT = np.ascontiguousarray(node_states.astype(NP_BT).T)  # [D, N]

    xstr = np.zeros((NCORES, P, TT * XCOLS), NP_F8)
    winstr = np.zeros((NCORES, P, SLOTS * 256), NP_BT)
    degs = np.zeros((NCORES, 1, SLOTS * P), np.float32)

    lane = np.arange(P, dtype=np.int64)
    for c in range(NCORES):
        v0a = np.zeros(TT * P, np.int64)
        v1a = np.zeros(TT * P, np.int64)
        la = np.full(TT * P, -1, np.int64)
        ea = np.full(TT * P, -1, np.int64)
        for j in range(SLOTS):
            w = int(assign[c, j])
            n = int(fills[w])
            b = int(base[j])
            ent = order[starts[w] : starts[w] + n]
            v0a[b * P : b * P + n] = ev0[ent]
            v1a[b * P : b * P + n] = ev1[ent]
            la[b * P : b * P + n] = dst[ent] - w * P
            ea[b * P : b * P + n] = eid[ent]
            winstr[c][:, j * 256 : j * 256 + 128] = nsbT[:, w * P : (w + 1) * P]
            winstr[c][:, j * 256 + 128 : j * 256 + 256] = nsbT[
                :, (w ^ pw) * P : ((w ^ pw) + 1) * P
            ]
            degs[c][0, j * P : (j + 1) * P] = np.bincount(
                la[b * P : b * P + n], minlength=P
            )

        # tile-major component streams, then interleave at 4-tile-group
        # granularity: [v0 | v1 | edge | onehot], each [P, ng*128] per group
        v0T = np.zeros((P, TT * P), NP_F8)
        v0T[:, :] = ns8[v0a].T
        v1T = np.zeros((P, TT * P), NP_F8)
        v1T[:, :] = ns8[v1a].T
        ef = ed8[np.clip(ea, 0, E - 1)]
        ef[ea < 0] = 0
        edT = np.zeros((P, TT * P), NP_F8)
        edT[0:ED, :] = ef.T
        ohT = np.ascontiguousarray(
            (la.reshape(TT, P)[:, :, None] == lane[None, None, :])
            .astype(NP_F8)
            .transpose(1, 0, 2)
            .reshape(P, TT * P)
        )
        xc = xstr[c]
        for j in range(SLOTS):
            cj = int(C[j])
            bj = int(base[j])
            for g0 in range(0, cj, 4):
                ng = min(4, cj - g0)
                s = (bj + g0) * P
                d = (bj + g0) * XCOLS
                w = ng * P
                xc[:, d : d + w] = v0T[:, s : s + w]
                xc[:, d + w : d + 2 * w] = v1T[:, s : s + w]
                xc[:, d + 2 * w : d + 3 * w] = edT[:, s : s + w]
                xc[:, d + 3 * w : d + 4 * w] = ohT[:, s : s + w]

    layout = {
        "N": N,
        "E": E,
        "NW": NW,
        "SLOTS": SLOTS,
        "TT": TT,
        "C": [int(x) for x in C],
        "base": [int(x) for x in base],
        "assign": assign,
    }
    percore = {"xstr": xstr, "winstr": winstr, "degs": degs}
    return layout, percore


def _prep_consts(inputs):
    """Shared (replicated) weight/bias/constant tensors."""

    def f32(x):
        return np.asarray(x, np.float32)

    W1 = f32(inputs["mW1"])  # [320, 256]
    W2 = f32(inputs["mW2"])  # [256, 256]
    W3 = f32(inputs["mW3"])  # [256, 128]
    U1 = f32(inputs["uW1"])  # [384, 256]
    U2 = f32(inputs["uW2"])  # [256, 256]
    U3 = f32(inputs["uW3"])  # [256, 128]

    w1a = (W1[0:256] * SW).reshape(2, P, H).transpose(1, 0, 2).astype(NP_F8)
    w1bc = np.zeros((P, 2, H), np.float32)
    w1bc[0:ED, 0, :] = W1[256:320] * SW
    w1b = w1bc.astype(NP_F8)
    w2r = (W2 * SW).reshape(2, P, H).transpose(1, 0, 2).astype(NP_F8)
    wf = (W3 @ U1[D : D + M] * SW).reshape(2, P, U).transpose(1, 0, 2).astype(NP_F8)
    uw1w = np.stack([U1[0:D], U1[D + M : D + M + D]], axis=1).astype(NP_BT)
    uw2r = U2.reshape(2, P, U).transpose(1, 0, 2).astype(NP_BT)
    uw3r = U3.reshape(2, P, D).transpose(1, 0, 2).astype(NP_BT)

    def halves(b):  # [2P] -> [P, 2] (column h = half h)
        return f32(b).reshape(2, P).T.copy()

    zb = {
        k: bool(np.all(np.asarray(inputs[k]) == 0))
        for k in ("mb1", "mb2", "mb3", "ub1", "ub2", "ub3")
    }
    # mb3 folds through the update-MLP summed chunk as a per-node
    # degree-scaled rank-1 term: (uW1_mid^T mb3) outer deg[n]
    wm3u = (f32(inputs["mb3"]) @ U1[D : D + M]).reshape(1, U).astype(NP_BT)

    consts = {
        "w1a": w1a.reshape(P, 2 * H),
        "w1b": w1b.reshape(P, 2 * H),
        "w2r": w2r.reshape(P, 2 * H),
        "wf": wf.reshape(P, 2 * U),
        "uw1w": uw1w.reshape(P, 2 * U),
        "uw2r": uw2r.reshape(P, 2 * U),
        "uw3r": uw3r.reshape(P, 2 * D),
        "wm3u": wm3u,
        "mb1h": halves(inputs["mb1"]),
        "mb2h": halves(inputs["mb2"]),
        "ub1h": halves(inputs["ub1"]),
        "ub2h": halves(inputs["ub2"]),
        "ub3r": np.tile(f32(inputs["ub3"])[None, :], (P, 1)),
    }
    return consts, zb


# ---------------------------------------------------------------- kernel IR
class _Router:
    """Greedy least-finish-time assignment of elementwise ops, using
    measured CoreSim cost-model rates. Pool (gpsimd) cannot touch PSUM
    on real TRN2 hardware, so PSUM-input ops only go to ACT / DVE."""

    RATE = {"act": 0.833, "dve": 1.042, "pool": 0.834}
    INIT = {"act": 185.0, "dve": 125.0, "pool": 2.0}

    def __init__(self):
        self.load = {"act": 0.0, "dve": 0.0, "pool": 0.0}

    def pick(self, cols, kind="psum"):
        cands = ("act", "dve") if kind == "psum" else ("pool", "dve", "act")
        best, bt, bfin = None, None, None
        for e in cands:
            fin = self.load[e] + cols * self.RATE[e] + self.INIT[e]
            # bias: big ops amortize ACT's higher per-op init; small ops
            # suit DVE. Score = finish time + affinity correction.
            t = fin
            if kind == "psum":
                if e == "act" and cols >= 1024:
                    t -= 120.0
                if e == "dve" and cols < 512:
                    t -= 60.0
            if bt is None or t < bt:
                best, bt, bfin = e, t, fin
        self.load[best] = bfin
        return best


def _build(layout, zb=None):
    zb = zb or {}
    SLOTS = layout["SLOTS"]
    TT = layout["TT"]
    C = layout["C"]
    base = layout["base"]

    nc = bacc.Bacc(None, target_bir_lowering=False)

    xstr = nc.dram_tensor("xstr", [P, TT * XCOLS], F8, kind="ExternalInput")
    winstr = nc.dram_tensor("winstr", [P, SLOTS * 256], BT, kind="ExternalInput")
    degs = nc.dram_tensor("degs", [1, SLOTS * P], FT, kind="ExternalInput")
    w1a = nc.dram_tensor("w1a", [P, 2 * H], F8, kind="ExternalInput")
    w1b = nc.dram_tensor("w1b", [P, 2 * H], F8, kind="ExternalInput")
    w2r = nc.dram_tensor("w2r", [P, 2 * H], F8, kind="ExternalInput")
    wf = nc.dram_tensor("wf", [P, 2 * U], F8, kind="ExternalInput")
    uw1w = nc.dram_tensor("uw1w", [P, 2 * U], BT, kind="ExternalInput")
    uw2r = nc.dram_tensor("uw2r", [P, 2 * U], BT, kind="ExternalInput")
    uw3r = nc.dram_tensor("uw3r", [P, 2 * D], BT, kind="ExternalInput")
    wm3u = nc.dram_tensor("wm3u", [1, U], BT, kind="ExternalInput")
    mb1h = nc.dram_tensor("mb1h", [P, 2], FT, kind="ExternalInput")
    mb2h = nc.dram_tensor("mb2h", [P, 2], FT, kind="ExternalInput")
    ub1h = nc.dram_tensor("ub1h", [P, 2], FT, kind="ExternalInput")
    ub2h = nc.dram_tensor("ub2h", [P, 2], FT, kind="ExternalInput")
    ub3r = nc.dram_tensor("ub3r", [P, D], FT, kind="ExternalInput")
    out = nc.dram_tensor("out", [SLOTS * P, D], FT, kind="ExternalOutput")

    RELU = mybir.ActivationFunctionType.Relu
    COPY = mybir.ActivationFunctionType.Copy
    MULT = mybir.AluOpType.mult
    MAX = mybir.AluOpType.max
    ADD = mybir.AluOpType.add
    SUB = mybir.AluOpType.subtract
    DR = mybir.MatmulPerfMode.DoubleRow

    rt = _Router()

    with tile.TileContext(nc) as tc:
        with (
            tc.tile_pool(name="const", bufs=1) as cp,
            tc.tile_pool(name="xs", bufs=4) as xp,
            tc.tile_pool(name="wn", bufs=4) as wp,
            tc.tile_pool(name="h1p", bufs=6) as h1p,
            tc.tile_pool(name="h2p", bufs=6) as h2p,
            tc.tile_pool(name="upd", bufs=4) as up,
            tc.tile_pool(name="ps1", bufs=2, space="PSUM") as ps1p,
            tc.tile_pool(name="ps2", bufs=3, space="PSUM") as ps2p,
            tc.tile_pool(name="gps", bufs=1, space="PSUM") as gp,
        ):
            # ---- elementwise op emitters (engine-routed)
            def emit_act(out_ap, in_ap, cols, scale, relu, bias=None,
                         kind="psum"):
                if bias is not None:
                    # nonzero-bias path: ACT only (func(in*scale + bias))
                    rt.load["act"] += cols * rt.RATE["act"] + rt.INIT["act"]
                    nc.scalar.activation(
                        out_ap, in_ap, RELU if relu else COPY,
                        bias=bias, scale=scale,
                    )
                    return
                e = rt.pick(cols, kind)
                if e == "act":
                    nc.scalar.activation(out_ap, in_ap, RELU if relu else COPY,
                                         scale=scale)
                else:
                    eng = nc.vector if e == "dve" else nc.gpsimd
                    if relu:
                        eng.tensor_scalar(out_ap, in_ap, scale, 0.0, MULT, MAX)
                    elif scale == 1.0:
                        eng.tensor_copy(out_ap, in_ap)
                    else:
                        eng.tensor_scalar(out_ap, in_ap, scale, None, MULT)

            # ---------------- per-slot state
            slot_ctx = {}

            pair_ctx = {}

            def slot_loads(j):
                cj = C[j]
                bj = base[j]
                xb = xp.tile([P, cj * XCOLS], F8, tag="xb")
                nc.sync.dma_start(
                    xb[:], xstr[:, bj * XCOLS : (bj + cj) * XCOLS]
                )
                if j % 2 == 0:
                    # per-PAIR window/partner states, attention, G bank
                    wv = wp.tile([P, 2, 256], BT, tag="win")
                    nc.sync.dma_start(
                        wv[:],
                        winstr[:, j * 256 : (j + 2) * 256].rearrange(
                            "p (s x) -> p s x", s=2
                        ),
                    )
                    attn = up.tile([P, 2, P], BT, tag="attn")
                    nc.gpsimd.tensor_tensor(
                        out=attn[:], in0=wv[:, :, 0:128], in1=wv[:, :, 128:256],
                        op=SUB,
                    )
                    rt.load["pool"] += 256 * 2.0
                    gt = gp.tile([P, 512], FT, tag="g")
                    pair_ctx[j // 2] = dict(wv=wv, attn=attn, gt=gt)
                pc = pair_ctx[j // 2]
                slot_ctx[j] = dict(xb=xb, pc=pc, h1g={})

            slot_loads(0)

            # ---- load constants once (slot 0's big stream is issued first
            # by the driver below, before these)
            def cload(t, shape, dt_):
                sb = cp.tile([P, shape[0], shape[1]], dt_, name=t.name + "_sb")
                nc.sync.dma_start(
                    sb[:], t[:].rearrange("p (c h) -> p c h", c=shape[0])
                )
                return sb

            w1a_sb = cload(w1a, (2, H), F8)
            w1b_sb = cload(w1b, (2, H), F8)
            w2r_sb = cload(w2r, (2, H), F8)
            wf_sb = cload(wf, (2, U), F8)
            uw1_sb = cload(uw1w, (2, U), BT)
            uw2_sb = cload(uw2r, (2, U), BT)
            uw3_sb = cload(uw3r, (2, D), BT)
            mb1_sb = cp.tile([P, 2], FT)
            nc.sync.dma_start(mb1_sb[:], mb1h[:])
            mb2_sb = cp.tile([P, 2], FT)
            nc.sync.dma_start(mb2_sb[:], mb2h[:])
            ub1_sb = cp.tile([P, 2], FT)
            nc.sync.dma_start(ub1_sb[:], ub1h[:])
            ub2_sb = cp.tile([P, 2], FT)
            nc.sync.dma_start(ub2_sb[:], ub2h[:])
            ub3_sb = cp.tile([P, D], FT)
            nc.sync.dma_start(ub3_sb[:], ub3r[:])
            ub3d_sb = cp.tile([P, 2 * D], FT)
            nc.sync.dma_start(ub3d_sb[:, 0:D], ub3r[:])
            nc.sync.dma_start(ub3d_sb[:, D : 2 * D], ub3r[:])
            if not zb.get("mb3"):
                wm3_sb = cp.tile([1, U], BT)
                nc.sync.dma_start(wm3_sb[:], wm3u[:])
                deg_sb = cp.tile([1, SLOTS * P], FT)
                nc.sync.dma_start(deg_sb[:], degs[:])


            # ---------------- block stages (2-tile blocks; L1 in 4-tile
            # groups so its ReLU runs as one 1024-col op)
            def emit_L1mm(it):
                if it["b0"] % 4 != 0:
                    return
                j, b0 = it["j"], it["b0"]
                cj = C[j]
                ng = min(4, cj - b0)
                sc = slot_ctx[j]
                ne = ng * P
                gb = b0 * XCOLS
                xv = sc["xb"][:, gb : gb + 4 * ne]
                rhs1 = xv[:, 0 : 2 * ne].rearrange("p (c e) -> p c e", c=2)
                rhs2 = xv[:, 2 * ne : 4 * ne].rearrange("p (c e) -> p c e", c=2)
                ps = ps1p.tile([P, 2, 4 * P], FT, tag="l1")
                for h in range(2):
                    nc.tensor.matmul(
                        ps[:, h, :ne],
                        lhsT=w1a_sb[:, :, h * P : (h + 1) * P],
                        rhs=rhs1,
                        perf_mode=DR,
                        start=True,
                        stop=False,
                    )
                    nc.tensor.matmul(
                        ps[:, h, :ne],
                        lhsT=w1b_sb[:, :, h * P : (h + 1) * P],
                        rhs=rhs2,
                        perf_mode=DR,
                        start=False,
                        stop=True,
                    )
                it["ps1"] = ps
                it["ng"] = ng

            def emit_L1act(it):
                if it["b0"] % 4 != 0:
                    return
                ng = it["ng"]
                ne = ng * P
                ps = it["ps1"]
                h1 = h1p.tile([P, 2, 4 * P], F8, tag="h1")
                if zb.get("mb1"):
                    if ng == 4:
                        emit_act(h1[:], ps[:], 2 * ne, 1.0 / SW, True)
                    else:
                        emit_act(h1[:, :, :ne], ps[:, :, :ne], 2 * ne,
                                 1.0 / SW, True)
                else:
                    for h in range(2):
                        emit_act(h1[:, h, :ne], ps[:, h, :ne], ne, 1.0 / SW,
                                 True, bias=mb1_sb[:, h : h + 1])
                slot_ctx[it["j"]]["h1g"][it["b0"] // 4] = h1

            def emit_L2mm(it):
                nb = it["nb"]
                off = it["b0"] % 4
                h1 = slot_ctx[it["j"]]["h1g"][it["b0"] // 4]
                ps2 = ps2p.tile([P, 2, H], FT, tag="l2")
                for t in range(nb):
                    nc.tensor.matmul(
                        ps2[:, t, :],
                        lhsT=h1[:, :, (off + t) * P : (off + t + 1) * P],
                        rhs=w2r_sb[:],
                        perf_mode=DR,
                        start=True,
                        stop=True,
                    )
                it["ps2"] = ps2

            def emit_L2act(it):
                nb = it["nb"]
                ps2 = it["ps2"]
                h2 = h2p.tile([P, 2, H], F8, tag="h2")
                if zb.get("mb2"):
                    emit_act(h2[:, 0:nb, :], ps2[:, 0:nb, :], nb * H,
                             1.0 / SW, True)
                else:
                    h2v = h2[:].rearrange("p t (c n) -> p t c n", c=2)
                    p2v = ps2[:].rearrange("p t (c n) -> p t c n", c=2)
                    for t in range(nb):
                        for h in range(2):
                            emit_act(h2v[:, t, h, :], p2v[:, t, h, :], P,
                                     1.0 / SW, True,
                                     bias=mb2_sb[:, h : h + 1])
                it["h2"] = h2

            def emit_G(it):
                j, nb = it["j"], it["nb"]
                sc = slot_ctx[j]
                b0 = it["b0"]
                off = b0 % 4
                g0 = b0 - off
                ng = min(4, C[j] - g0)
                ohbase = g0 * XCOLS + 3 * ng * P + off * P
                ohv = sc["xb"][:, ohbase : ohbase + nb * P].rearrange(
                    "p (c e) -> p c e", c=nb
                )
                h2 = it["h2"]
                half = (j % 2) * 256
                gv = sc["pc"]["gt"][:, half : half + 256].rearrange(
                    "p (c n) -> p c n", c=2
                )
                sp = it["last"] and j % 2 == 1
                for hc in range(2):
                    # one zero-region (bank) holds 2 slots x 2 hc halves: only
                    # the very first matmul starts it (start=True marks the
                    # WHOLE bank pending-zero; later sub-regions' first writes
                    # self-zero via the pending flag).
                    st = it["first"] and hc == 0 and j % 2 == 0
                    if nb == 2:
                        nc.tensor.matmul(
                            gv[:, hc, :],
                            lhsT=h2[:, 0:2, hc * P : (hc + 1) * P],
                            rhs=ohv[:],
                            perf_mode=DR,
                            start=st,
                            stop=sp and hc == 1,
                            skip_group_check=True,
                        )
                    else:
                        nc.tensor.matmul(
                            gv[:, hc, :],
                            lhsT=h2[:, 0, hc * P : (hc + 1) * P],
                            rhs=ohv[:, 0, :],
                            start=st,
                            stop=sp and hc == 1,
                            skip_group_check=True,
                        )
                if it["last"] and j % 2 == 1:
                    gsb = up.tile([P, 2, 256], F8, tag="gsb")
                    emit_act(
                        gsb[:].rearrange("p hc (s n) -> p s hc n", s=2),
                        sc["pc"]["gt"][:].rearrange(
                            "p (s hc n) -> p s hc n", s=2, hc=2
                        ),
                        512,
                        1.0 / SW,
                        False,
                    )
                    sc["pc"]["gsb"] = gsb

            # ---------------- per-PAIR update MLP (3 phases over 2 slots)
            def emit_update(q, phase):
                if phase == 0:
                    _upd_p0(q)
                elif phase == 1:
                    _upd_p1(q)
                else:
                    _upd_p2(q)

            def _upd_p0(q):
                pc = pair_ctx[q]
                wv, attn, gsb = pc["wv"], pc["attn"], pc["gsb"]
                u1ps = ps2p.tile([P, 2, 256], FT, tag="l2")
                for h in range(2):
                    nc.tensor.matmul(
                        u1ps[:, h, :],
                        lhsT=uw1_sb[:, 0, h * P : (h + 1) * P],
                        rhs=wv[:, :, 0:128],
                        start=True,
                        stop=False,
                    )
                    nc.tensor.matmul(
                        u1ps[:, h, :],
                        lhsT=uw1_sb[:, 1, h * P : (h + 1) * P],
                        rhs=attn[:],
                        start=False,
                        stop=False,
                    )
                    if not zb.get("mb3"):
                        nc.tensor.matmul(
                            u1ps[:, h, :],
                            lhsT=wm3_sb[:, h * P : (h + 1) * P],
                            rhs=deg_sb[:, q * 256 : (q + 1) * 256],
                            start=False,
                            stop=False,
                        )
                    nc.tensor.matmul(
                        u1ps[:, h, :],
                        lhsT=wf_sb[:, :, h * P : (h + 1) * P],
                        rhs=gsb[:],
                        perf_mode=DR,
                        start=False,
                        stop=True,
                    )
                u1 = up.tile([P, 2, 256], BT, tag="u1")
                if zb.get("ub1"):
                    emit_act(u1[:].rearrange("p c n -> p (c n)"),
                             u1ps[:].rearrange("p c n -> p (c n)"),
                             512, 1.0, True)
                else:
                    for h in range(2):
                        emit_act(u1[:, h, :], u1ps[:, h, :], 256, 1.0, True,
                                 bias=ub1_sb[:, h : h + 1])
                pc["u1"] = u1

            def _upd_p1(q):
                pc = pair_ctx[q]
                u1 = pc["u1"]
                u2ps = ps2p.tile([P, 2, 256], FT, tag="l2")
                for h in range(2):
                    for cc in range(2):
                        nc.tensor.matmul(
                            u2ps[:, h, :],
                            lhsT=uw2_sb[:, cc, h * P : (h + 1) * P],
                            rhs=u1[:, cc, :],
                            start=(cc == 0),
                            stop=(cc == 1),
                        )
                u2 = up.tile([P, 2, 256], BT, tag="u2")
                if zb.get("ub2"):
                    emit_act(u2[:].rearrange("p c n -> p (c n)"),
                             u2ps[:].rearrange("p c n -> p (c n)"),
                             512, 1.0, True)
                else:
                    for h in range(2):
                        emit_act(u2[:, h, :], u2ps[:, h, :], 256, 1.0, True,
                                 bias=ub2_sb[:, h : h + 1])
                pc["u2"] = u2

            def _upd_p2(q):
                pc = pair_ctx[q]
                u2 = pc["u2"]
                ops_ = ps2p.tile([P, 2, P], FT, tag="l2")
                for s in range(2):
                    for cc in range(2):
                        nc.tensor.matmul(
                            ops_[:, s, :],
                            lhsT=u2[:, cc, s * P : (s + 1) * P],
                            rhs=uw3_sb[:, cc, :],
                            start=(cc == 0),
                            stop=(cc == 1),
                        )
                osb = up.tile([P, 2, P], FT, tag="osb")
                if zb.get("ub3"):
                    emit_act(osb[:].rearrange("p s n -> p (s n)"),
                             ops_[:].rearrange("p s n -> p (s n)"),
                             256, 1.0, False)
                else:
                    nc.vector.tensor_tensor(
                        out=osb[:].rearrange("p s n -> p (s n)"),
                        in0=ops_[:].rearrange("p s n -> p (s n)"),
                        in1=ub3_sb[:].rearrange("p d -> p () d").broadcast(1, 2)
                        if False else ub3d_sb[:],
                        op=ADD,
                    )
                    rt.load["dve"] += 256 * 1.042 + 125
                for s in range(2):
                    nc.sync.dma_start(
                        out[(2 * q + s) * P : (2 * q + s + 1) * P, :],
                        osb[:, s, :],
                    )

            # ---------------- software-pipelined driver
            work = []
            for j in range(SLOTS):
                cj = C[j]
                for b0 in range(0, cj, 2):
                    nb = min(2, cj - b0)
                    work.append(
                        dict(j=j, b0=b0, nb=nb,
                             first=(b0 == 0), last=(b0 + nb == cj))
                    )

            n = len(work)
            _off = STAGE_OFFSETS
            stages = [(_off[0], emit_L1mm), (_off[1], emit_L1act),
                      (_off[2], emit_L2mm), (_off[3], emit_L2act),
                      (_off[4], emit_G)]
            upd_q = []
            for i in range(n + 24):
                while upd_q and upd_q[0][0] <= i:
                    _, jq, ph = upd_q.pop(0)
                    emit_update(jq, ph)
                for off, emit in stages:
                    k = i - off
                    if 0 <= k < n:
                        if off == 0 and work[k]["first"]:
                            jn = work[k]["j"] + 1
                            if jn < SLOTS:
                                slot_loads(jn)
                        emit(work[k])
                        if (off == _off[4] and work[k]["last"]
                                and work[k]["j"] % 2 == 1):
                            qd = work[k]["j"] // 2
                            upd_q.append((i + 4, qd, 0))
                            upd_q.append((i + 8, qd, 1))
                            upd_q.append((i + 11, qd, 2))

    nc.finalize()
    return nc


# ---------------------------------------------------------------- execution
_cache = {}


def _core_map(percore, consts, ns_cast, c):
    m = {
        "xstr": percore["xstr"][c],
        "winstr": percore["winstr"][c],
        "degs": percore["degs"][c],
    }
    m.update(consts)
    return m


def _run(inputs, trace=False):
    import time

    t0 = time.time()
    layout, percore = _preprocess(
        inputs["node_states"], inputs["edges"], inputs["vertices"]
    )
    consts, zb = _prep_consts(inputs)
    print(f"[kernel] preprocess {time.time() - t0:.1f}s TT={layout['TT']}",
          flush=True)

    t0 = time.time()
    key = (layout["TT"], tuple(layout["C"]), layout["N"],
           tuple(sorted(zb.items())))
    if key not in _cache:
        _cache[key] = _build(layout, zb)
    nc = _cache[key]
    print(f"[kernel] build {time.time() - t0:.1f}s insts={len(nc.inst_map)}",
          flush=True)
    t0 = time.time()

    in_maps = [_core_map(percore, consts, None, c) for c in range(NCORES)]
    res = run_bass_kernel_spmd(nc, in_maps, core_ids=list(range(NCORES)),
                               trace=trace)
    print(f"[kernel] compile+run {time.time() - t0:.1f}s", flush=True)

    N = layout["N"]
    outg = np.zeros((N, D), np.float32)
    assign = layout["assign"]
    for c in range(NCORES):
        oc = np.asarray(res.results[c]["out"])
        for j in range(layout["SLOTS"]):
            w = int(assign[c, j])
            outg[w * P : (w + 1) * P, :] = oc[j * P : (j + 1) * P, :]
    return outg, res.exec_time_ns


def kernel(**inputs) -> np.ndarray:
    out, _ = _run(inputs, trace=False)
    return out
